# revision 21
# baseline (speedup 1.0000x reference)
"""Trainium2 Bass kernel for nn_HNet3_74801150427700 (topk_masking).

ref:  x = out.view(-1, 8); v = sort(x,1)[:, 3]  (4th smallest)
      y = softmax(x, 1) * (x > v)

Since exp is monotone, the rank-3 selection and the mask commute with
exp: run everything in the exp domain on fp16 "k-planes".

Per tile [128, 8192] fp32 (1024 groups of 8 per partition):
  ACT:    E_k = exp(x[:, k::8]) -> fp16 planes (one fused op)
  GPSIMD: softmax denominator: 3-level pair-sum add tree on the exp
          planes -> S32 (contiguous plane operands only; strided APs
          are very slow on the Q7 cores)
  DVE:    27-op min/max selection network on planes -> V = 4th-smallest
          exp; then per-plane M_k = E_k > V, Q_k = E_k * R16,
          G_k = M_k * Q_k. All ops single-free-dim contiguous fp16:
          only fully-packed 1D APs engage the 2x DVE mode on HW.
  ACT:    y[:, k::8] = fp32(G_k) back into the x tile (one fused op);
          DMA out

Sharding: rows split evenly across 8 cores (data parallel).
fp16 effects (mask flips on fp16-collisions ~0.09% of groups, value
rounding ~5e-4): measured rel err ~7.7e-3 vs fp32 reference.
"""

import numpy as np

_NCORES = 8
_ROWS = 8388608
_K = 8
_P = 128
_C = 8192
_ELEMS_PER_CORE = _ROWS * _K // _NCORES
_NT = _ELEMS_PER_CORE // (_P * _C)          # 8

_nc_cache = {}


def _build(nt=_NT, c=_C, reps=1, mqg="plane", act="fused", sums="pool"):
    """mqg/act: "plane" (per-plane ops) or "fused"; sums: "pool"|"dve"."""
    import concourse.bacc as bacc
    import concourse.mybir as mybir
    from contextlib import ExitStack
    from concourse.tile import TileContext

    f32 = mybir.dt.float32
    f16 = mybir.dt.float16
    AF = mybir.ActivationFunctionType
    OP = mybir.AluOpType
    k = _K
    f = c // k                               # plane width (1024)

    nc = bacc.Bacc(None, target_bir_lowering=False)
    xd = nc.declare_dram_parameter("x", [nt, _P, c], f32, isOutput=False)
    yd = nc.declare_dram_parameter("y", [nt, _P, c], f32, isOutput=True)

    with TileContext(nc) as tc, ExitStack() as ctx:
        xp = ctx.enter_context(tc.tile_pool(name="xp", bufs=2))
        ep = ctx.enter_context(tc.tile_pool(name="ep", bufs=2))
        sp1 = ctx.enter_context(tc.tile_pool(name="sp1", bufs=2))
        smp = ctx.enter_context(tc.tile_pool(name="smp", bufs=2))
        rp = ctx.enter_context(tc.tile_pool(name="rp", bufs=2))

        def pl(tile, i):
            return tile[:, i * f:(i + 1) * f]

        def body():
            for t in range(nt):
                xt = xp.tile([_P, c], f32)
                nc.sync.dma_start(out=xt[:], in_=xd[t])

                # exp planes (fp16)
                Ep = ep.tile([_P, c], f16)
                xkg = xt[:].rearrange("p (g k) -> p k g", k=k)
                ekg = Ep[:].rearrange("p (k g) -> p k g", k=k)
                x3 = xt[:].rearrange("p (g k) -> p g k", k=k)
                if act == "fused":
                    nc.scalar.activation(ekg, xkg, AF.Exp)
                else:
                    for i in range(k):
                        nc.scalar.activation(
                            Ep[:, i * f:(i + 1) * f], x3[:, :, i], AF.Exp)

                # softmax denominator: pair-tree adds with contiguous
                # plane operands (strided APs are slow on Pool)
                S32 = rp.tile([_P, f], f32, tag="S32")
                sm = smp.tile([_P, 6 * f], f16, tag="sm")
                eng = nc.gpsimd if sums == "pool" else nc.vector
                for i in range(4):
                    eng.tensor_tensor(
                        pl(sm, i), pl(Ep, 2 * i), pl(Ep, 2 * i + 1), op=OP.add)
                eng.tensor_tensor(
                    pl(sm, 4), pl(sm, 0), pl(sm, 1), op=OP.add)
                eng.tensor_tensor(
                    pl(sm, 5), pl(sm, 2), pl(sm, 3), op=OP.add)
                eng.tensor_tensor(
                    S32[:], pl(sm, 4), pl(sm, 5), op=OP.add)

                # selection network: 27 single-plane contiguous fp16 DVE
                # ops (only fully-packed 1-free-dim APs hit 2x on HW) on
                # one scratch tile S of 16 plane slots
                S = sp1.tile([_P, 2 * c], f16)
                TT = nc.vector.tensor_tensor

                def sl(i):
                    return S[:, i * f:(i + 1) * f]

                # L1: lo -> S[0:4], hi -> S[4:8]
                for i in range(4):
                    TT(sl(i), pl(Ep, 2 * i), pl(Ep, 2 * i + 1), op=OP.min)
                for i in range(4):
                    TT(sl(4 + i), pl(Ep, 2 * i), pl(Ep, 2 * i + 1), op=OP.max)
                # L2: ab1p -> S[8:10], ab2p -> S[10:12], ab0 -> S[12:14],
                #     ab3 -> S[14:16]
                TT(sl(12), sl(0), sl(1), op=OP.min)
                TT(sl(13), sl(2), sl(3), op=OP.min)
                TT(sl(8), sl(0), sl(1), op=OP.max)
                TT(sl(9), sl(2), sl(3), op=OP.max)
                TT(sl(10), sl(4), sl(5), op=OP.min)
                TT(sl(11), sl(6), sl(7), op=OP.min)
                TT(sl(14), sl(4), sl(5), op=OP.max)
                TT(sl(15), sl(6), sl(7), op=OP.max)
                # ab1 = min(ab1p, ab2p) -> S[0:2]; ab2 = max -> S[2:4]
                TT(sl(0), sl(8), sl(10), op=OP.min)
                TT(sl(1), sl(9), sl(11), op=OP.min)
                TT(sl(2), sl(8), sl(10), op=OP.max)
                TT(sl(3), sl(9), sl(11), op=OP.max)
                # m-layer, p-layer, V
                TT(sl(4), sl(12), sl(13), op=OP.max)   # m3
                TT(sl(5), sl(0), sl(1), op=OP.max)     # m1
                TT(sl(6), sl(14), sl(15), op=OP.min)   # m2
                TT(sl(7), sl(2), sl(3), op=OP.min)     # m4
                TT(sl(8), sl(6), sl(5), op=OP.min)     # p3
                TT(sl(9), sl(7), sl(4), op=OP.max)     # p4
                TT(sl(10), sl(8), sl(9), op=OP.min)    # V -> S[10]

                # M/Q/G: M -> S[0:8] before the reciprocal so the GPSIMD
                # sum tree has time to land; Q -> S[8:16] clobbers V after
                # M (DVE in-order); G -> Ep in place.
                if mqg == "plane":
                    # per-plane contiguous ops (V / R16 same-shape operands)
                    for i in range(k):
                        TT(sl(i), pl(Ep, i), sl(10), op=OP.is_gt)
                else:
                    Mkg = S[:, 0:c].rearrange("p (k g) -> p k g", k=k)
                    Vb = sl(10).unsqueeze(1).broadcast_to([_P, k, f])
                    TT(Mkg, ekg, Vb, op=OP.is_gt)

                R32 = rp.tile([_P, f], f32, tag="R32")
                nc.vector.reciprocal_approx_fast(R32[:], S32[:])
                R16 = rp.tile([_P, f], f16, tag="R16")
                nc.scalar.activation(R16[:], R32[:], AF.Copy)

                if mqg == "plane":
                    for i in range(k):
                        TT(sl(8 + i), pl(Ep, i), R16[:], op=OP.mult)
                    for i in range(k):
                        TT(pl(Ep, i), sl(i), sl(8 + i), op=OP.mult)
                else:
                    Qkg = S[:, c:2 * c].rearrange("p (k g) -> p k g", k=k)
                    Mkg = S[:, 0:c].rearrange("p (k g) -> p k g", k=k)
                    Rb = R16[:].unsqueeze(1).broadcast_to([_P, k, f])
                    TT(Qkg, ekg, Rb, op=OP.mult)
                    TT(ekg, Mkg, Qkg, op=OP.mult)

                # convert back to interleaved fp32, reusing xt as the
                # output tile
                if act == "fused":
                    nc.scalar.activation(xkg, ekg, AF.Copy)
                else:
                    for i in range(k):
                        nc.scalar.activation(
                            x3[:, :, i], Ep[:, i * f:(i + 1) * f], AF.Copy)
                nc.sync.dma_start(out=yd[t], in_=xt[:])

        if reps > 1:
            with tc.For_i(0, reps, 1):
                body()
        else:
            body()
    nc.finalize()
    return nc


def _get_nc(nt=_NT, c=_C, reps=1, **kw):
    key = (nt, c, reps, tuple(sorted(kw.items())))
    if key not in _nc_cache:
        _nc_cache[key] = _build(nt, c, reps, **kw)
    return _nc_cache[key]


def _shard_in_map(flat_core_x):
    """flat_core_x: flat fp32 array of this core's elements -> input map."""
    return {"x": flat_core_x.reshape(_NT, _P, _C)}


def _unshard_out_map(res):
    """result map for one core -> flat fp32 array of this core's outputs."""
    return np.asarray(res["y"]).reshape(-1)


def _run(x_np, trace=False):
    """x_np: [ROWS, 8] fp32. Returns (y [ROWS,8] fp32, exec_time_ns|None)."""
    from concourse.bass_utils import run_bass_kernel_spmd

    nc = _get_nc()
    xs = np.ascontiguousarray(x_np, dtype=np.float32).reshape(_NCORES, -1)
    in_maps = [_shard_in_map(xs[i]) for i in range(_NCORES)]
    out = run_bass_kernel_spmd(nc, in_maps, list(range(_NCORES)), trace=trace)
    y = np.stack([_unshard_out_map(out.results[i]) for i in range(_NCORES)])
    return y.reshape(_ROWS, _K), out.exec_time_ns


def _make_sharded_fn(nc):
    import jax
    from jax.experimental.shard_map import shard_map
    from jax.sharding import Mesh, NamedSharding, PartitionSpec

    import concourse.mybir as mybir
    from concourse.bass2jax import (
        _bass_exec_p,
        install_neuronx_cc_hook,
        partition_id_tensor,
    )

    install_neuronx_cc_hook()
    pname = nc.partition_id_tensor.name if nc.partition_id_tensor else None

    in_names, out_names, out_avals, zero_outs = [], [], [], []
    for alloc in nc.m.functions[0].allocations:
        if not isinstance(alloc, mybir.MemoryLocationSet):
            continue
        name = alloc.memorylocations[0].name
        if alloc.kind == "ExternalInput":
            if name != pname:
                in_names.append(name)
        elif alloc.kind == "ExternalOutput":
            out_names.append(name)
            shape = tuple(alloc.tensor_shape)
            dtype = mybir.dt.np(alloc.dtype)
            out_avals.append(jax.core.ShapedArray(shape, dtype))
            zero_outs.append(np.zeros(shape, dtype))
    n_params = len(in_names)
    all_in_names = in_names + out_names
    if pname is not None:
        all_in_names = all_in_names + [pname]

    def _body(*args):
        operands = list(args)
        if pname is not None:
            operands.append(partition_id_tensor())
        outs = _bass_exec_p.bind(
            *operands,
            out_avals=tuple(out_avals),
            in_names=tuple(all_in_names),
            out_names=tuple(out_names),
            lowering_input_output_aliases=(),
            sim_require_finite=True,
            sim_require_nnan=True,
            nc=nc,
        )
        return tuple(outs)

    devices = jax.devices()[:_NCORES]
    mesh = Mesh(np.asarray(devices), ("core",))
    spec = PartitionSpec("core")
    n_outs = len(out_names)
    sharded = jax.jit(
        shard_map(
            _body,
            mesh=mesh,
            in_specs=(spec,) * (n_params + n_outs),
            out_specs=(spec,) * n_outs,
            check_rep=False,
        ),
        keep_unused=True,
    )
    sh = NamedSharding(mesh, spec)
    return sharded, sh, zero_outs


def _bench(x_np, reps_hi=17, iters=8, **kw):
    """Device-resident per-pass time via slope between a reps=1 program
    and a hardware-looped reps=reps_hi program."""
    import time
    import jax

    xs = np.ascontiguousarray(x_np, dtype=np.float32).reshape(
        _NCORES * _NT, _P, _C
    )
    progs = {}
    for reps in (1, reps_hi):
        nc = _get_nc(reps=reps, **kw)
        sharded, sh, zero_outs = _make_sharded_fn(nc)
        xin = jax.device_put(xs, sh)
        zin = [
            jax.device_put(
                np.zeros((_NCORES * z.shape[0], *z.shape[1:]), z.dtype), sh
            )
            for z in zero_outs
        ]
        outs = sharded(xin, *zin)
        jax.block_until_ready(outs)
        progs[reps] = (sharded, xin, zin, outs)

    y1 = np.asarray(progs[1][3][0]).reshape(_ROWS, _K)

    def timed(reps):
        sharded, xin, zin, _ = progs[reps]
        t0 = time.perf_counter()
        o = sharded(xin, *zin)
        jax.block_until_ready(o)
        return time.perf_counter() - t0

    timed(1), timed(reps_hi)
    diffs = []
    for _ in range(iters):
        t1 = timed(1)
        tH = timed(reps_hi)
        diffs.append(tH - t1)
    diffs.sort()
    med = diffs[len(diffs) // 2]
    return y1, med / (reps_hi - 1)


def kernel(out, num_per_group):
    x = np.asarray(out, dtype=np.float32)
    assert x.shape == (_ROWS, _K), x.shape
    assert int(num_per_group) == _K
    y, _ = _run(x)
    return y
